# revision 25
# baseline (speedup 1.0000x reference)
"""Multi-head attention (B=2, T=2048, C=1024, H=16, D=64) on 8 TRN2 cores.

Sharding: core = b*4 + hg  -> batch b in {0,1}, head-group hg covers heads
[4hg, 4hg+4). Each core computes qkv projection for its 4 heads, attention,
and a partial out-projection [T, C]; host sums the 4 partials per batch and
adds b_out.

Kernel math (per core, all matmuls fp32r, PSUM accumulate fp32):
  xT [C, T] (host pre-transposed)    QT/KT pair tiles [128=2 heads, T]
  scoresT[k, q] = KT-lhsT @ QT-rhs   (per head, d=64 contraction,
                                      head pairs packed in PE row groups)
  eT = Exp(scoresT / 8)              (ACT, PSUM->SBUF)
  attnT[d+1, q] += Vhat-lhsT @ eT    (Vhat has an appended ones column ->
                                      row 64 accumulates softmax denominators)
  r = Exp(-Ln(sums))                 (ACT; Ln+Exp share one table set)
  attnN = attnU * bcast(r)           (DVE; r broadcast via 0-stride DMA)
  out[t, :] += attnN-lhsT @ Wout-rhs (partial, summed on host)
"""
import os
import numpy as np

import concourse.bacc as bacc
import concourse.tile as tile
import concourse.bass as bass
import concourse.mybir as mybir

F32 = mybir.dt.float32
F32R = mybir.dt.float32r
BF16 = mybir.dt.bfloat16
F16 = mybir.dt.float16
AF = mybir.ActivationFunctionType

B, T, C, H, D = 2, 2048, 1024, 16, 64
NCORES = 8
HG = 4          # head groups = cores per batch
HPC = H // HG   # heads per core = 4
QC = 512        # q-chunk columns
NQC = T // QC   # 2
CT = C // 128   # 8 contraction tiles for projections
KT = T // 128   # 16 key tiles
MT = T // 128   # 16 out-row tiles


def build(loops: int = 1, debug: bool = False):
    nc = bacc.Bacc("TRN2", target_bir_lowering=False, debug=False)
    xt_d = nc.dram_tensor("xt", [C, T], F32R, kind="ExternalInput").ap()
    wqkv_d = nc.dram_tensor("wqkv", [C, 768], F32R, kind="ExternalInput").ap()
    bqk_d = nc.dram_tensor("bqk", [512], F32, kind="ExternalInput").ap()
    bv_d = nc.dram_tensor("bv", [256], F32, kind="ExternalInput").ap()
    wout_d = nc.dram_tensor("wout", [256, C], BF16, kind="ExternalInput").ap()
    outp_d = nc.dram_tensor("outp", [T, C], F32, kind="ExternalOutput").ap()
    rscr_d = nc.dram_tensor("rscr", [64, QC], F32).ap()
    if debug:
        dbg_qk = nc.dram_tensor("dbg_qk", [128, T], BF16, kind="ExternalOutput").ap()
        dbg_vh = nc.dram_tensor("dbg_vh", [128, KT * HPC * 65], BF16, kind="ExternalOutput").ap()
        dbg_sum = nc.dram_tensor("dbg_sum", [64, QC], F32, kind="ExternalOutput").ap()
        dbg_au = nc.dram_tensor("dbg_au", [128, T], BF16, kind="ExternalOutput").ap()

    with tile.TileContext(nc) as tc:
        from contextlib import ExitStack, nullcontext
        loop_cm = (tc.For_i(0, loops, hint_engines=(
            mybir.EngineType.PE, mybir.EngineType.Activation,
            mybir.EngineType.DVE, mybir.EngineType.SP,
            mybir.EngineType.Pool)) if loops > 1 else nullcontext())
        with loop_cm:
            with ExitStack() as stk:
                sp = stk.enter_context(tc.tile_pool(name="sp", bufs=1))
                dp = stk.enter_context(tc.tile_pool(name="dp", bufs=1))
                psp = stk.enter_context(
                    tc.tile_pool(name="psp", bufs=2, space="PSUM"))
                pap = stk.enter_context(
                    tc.tile_pool(name="pap", bufs=2, space="PSUM"))

                # ---------- persistent tiles ----------
                qk_sb = [sp.tile([128, T], BF16, tag=f"qk{j}", name=f"qk{j}")
                         for j in range(4)]
                vhat = sp.tile([128, KT * HPC * 65], BF16, tag="vh", name="vh")
                wout_sb = [sp.tile([128, C], BF16, tag=f"wo{k}", name=f"wo{k}")
                           for k in range(2)]
                attnU = [sp.tile([128, T], BF16, tag=f"au{p}", name=f"au{p}")
                         for p in range(2)]
                # softmax denominators: rows (h*NQC + qc), partition-parallel
                sums_all = sp.tile([64, QC], F32, tag="sum")
                rinv = sp.tile([64, QC], F32, tag="rin")

                nc.sync.dma_start(wout_sb[0][:], wout_d[0:128, :])
                nc.sync.dma_start(wout_sb[1][:], wout_d[128:256, :])

                # ---------- loads ----------
                wq_sb = [sp.tile([128, 768], F32R, tag=f"wq{i}", name=f"wq{i}")
                         for i in range(CT)]
                bqk_sb = [sp.tile([128, 1], F32, tag=f"bq{j}", name=f"bq{j}")
                          for j in range(4)]
                bv_sb = sp.tile([128, 256], F32, tag="bv")
                for j in range(4):
                    nc.sync.dma_start(
                        bqk_sb[j][:], bqk_d[j * 128:(j + 1) * 128].unsqueeze(1))
                nc.sync.dma_start(bv_sb[:], bv_d[:].partition_broadcast(128))

                # ---------- projections (streamed x^T quarters) ----------
                def proj_pass(jts, do_v, pid):
                    for tch in range(NQC):
                        xt_sb = [dp.tile([128, QC], F32R, tag=f"xt{i}",
                                         name=f"xt{pid}_{tch}_{i}", bufs=2)
                                 for i in range(CT)]
                        for i in range(CT):
                            if pid == 0 and tch == 0:
                                nc.sync.dma_start(
                                    wq_sb[i][:],
                                    wqkv_d[i * 128:(i + 1) * 128, :])
                            nc.sync.dma_start(
                                xt_sb[i][:],
                                xt_d[i * 128:(i + 1) * 128,
                                     tch * QC:(tch + 1) * QC])
                        for jt in jts:
                            pq = psp.tile([128, QC], F32, tag="s", name="pq")
                            for ct in range(CT):
                                nc.tensor.matmul(
                                    pq[:],
                                    wq_sb[ct][:, jt * 128:(jt + 1) * 128],
                                    xt_sb[ct][:],
                                    start=(ct == 0), stop=(ct == CT - 1))
                            nc.vector.tensor_scalar_add(
                                qk_sb[jt][:, tch * QC:(tch + 1) * QC],
                                pq[:], bqk_sb[jt][:])
                        if not do_v:
                            continue
                        for tl in range(QC // 128):
                            tt = tch * (QC // 128) + tl
                            pv = psp.tile([128, 256], F32, tag="s", name="pv")
                            for ct in range(CT):
                                nc.tensor.matmul(
                                    pv[:, 0:256],
                                    xt_sb[ct][:, tl * 128:(tl + 1) * 128],
                                    wq_sb[ct][:, 512:768],
                                    start=(ct == 0), stop=(ct == CT - 1))
                            v3 = vhat[:, tt * 260:(tt + 1) * 260].rearrange(
                                "p (h x) -> p h x", h=HPC)
                            bv3 = bv_sb[:].rearrange("p (h x) -> p h x", h=HPC)
                            nc.vector.tensor_scalar(
                                v3[:, :, 64:65], bv3[:, :, 0:1], 0.0, 1.0,
                                op0=mybir.AluOpType.mult,
                                op1=mybir.AluOpType.add)
                            nc.vector.tensor_tensor(
                                v3[:, :, 0:64],
                                pv[:, 0:256].rearrange("p (h x) -> p h x", h=HPC),
                                bv_sb[:].rearrange("p (h x) -> p h x", h=HPC),
                                op=mybir.AluOpType.add)

                def attention(pg, side_setup=None, side_step=None):
                    qt_t, kt_t = qk_sb[2 * pg], qk_sb[2 * pg + 1]
                    for qc in range(NQC):
                        if side_setup is not None:
                            side_setup(qc)
                        pa = [pap.tile([65, QC], F32, tag="a", name="pa")
                              for _ in range(2)]
                        def attn_mms(kt, e_pair):
                            for a in range(2):
                                h = 2 * pg + a
                                nc.tensor.matmul(
                                    pa[a][:],
                                    vhat[:, kt * 260 + h * 65:
                                         kt * 260 + (h + 1) * 65],
                                    e_pair[a],
                                    start=(kt == 0), stop=(kt == KT - 1))

                        e_prev = None
                        for kt in range(KT):
                            ph = psp.tile([128, 2 * QC], F32, tag="s",
                                          name="ph", bufs=2)
                            for a in range(2):
                                lo, hi = a * 64, a * 64 + 64
                                s0 = qc * QC
                                nc.tensor.matmul(
                                    ph[:, a * QC:(a + 1) * QC],
                                    kt_t[lo:hi, kt * 128:(kt + 1) * 128],
                                    qt_t[lo:hi, s0:s0 + QC],
                                    start=True, stop=True)
                            if e_prev is not None:
                                attn_mms(kt - 1, e_prev)
                            if side_step is not None:
                                side_step(qc, kt)
                            efull = dp.tile([128, 2 * QC], BF16, tag="e",
                                            name="e", bufs=4)
                            nc.scalar.activation(efull[:], ph[:],
                                                 AF.Exp, scale=0.125)
                            e_cur = [efull[:, a * QC:(a + 1) * QC]
                                     for a in range(2)]
                            e_prev = e_cur
                        attn_mms(KT - 1, e_prev)
                        for a in range(2):
                            h = 2 * pg + a
                            row = pg * 32 + a * NQC + qc
                            sstg = dp.tile([65, QC], F32, tag="sst", name="sstg",
                                           bufs=2)
                            nc.vector.tensor_copy(sstg[64:65, :],
                                                  pa[a][64:65, :])
                            nc.sync.dma_start(sums_all[row:row + 1, :],
                                              sstg[64:65, :])
                            nc.vector.tensor_copy(
                                attnU[pg][a * 64:(a + 1) * 64,
                                          qc * QC:(qc + 1) * QC],
                                pa[a][0:64, :])

                    # normalize this pair now (overlaps next pair's attention)
                    r0 = pg * 32
                    nc.vector.reciprocal(rinv[r0:r0 + 2 * NQC, :],
                                         sums_all[r0:r0 + 2 * NQC, :])
                    nc.sync.dma_start(rscr_d[r0:r0 + 2 * NQC, :],
                                      rinv[r0:r0 + 2 * NQC, :])
                    for qc in range(NQC):
                        rb = dp.tile([128, QC], F32, tag="rb", name="rb", bufs=2)
                        for a in range(2):
                            row = pg * 32 + a * NQC + qc
                            nc.sync.dma_start(
                                rb[a * 64:(a + 1) * 64, :],
                                rscr_d[row, :].partition_broadcast(64))
                        nc.vector.tensor_tensor(
                            attnU[pg][:, qc * QC:(qc + 1) * QC],
                            attnU[pg][:, qc * QC:(qc + 1) * QC],
                            rb[:],
                            op=mybir.AluOpType.mult)


                # V + pair-0 QK first so attention(0) starts early; pair-1 QK
                # projections are interleaved into attention(0)'s kt loop
                proj_pass([0, 1], True, 0)

                side = {"xt": None, "pq": None}

                def side_setup(qc):
                    side["xt"] = [dp.tile([128, QC], F32R, tag=f"xt{i}",
                                          name=f"xts_{qc}_{i}", bufs=2)
                                  for i in range(CT)]
                    for i in range(CT):
                        nc.sync.dma_start(
                            side["xt"][i][:],
                            xt_d[i * 128:(i + 1) * 128,
                                 qc * QC:(qc + 1) * QC])

                def side_step(qc, kt):
                    jt = 2 + kt // CT
                    ct = kt % CT
                    if ct == 0:
                        side["pq"] = psp.tile([128, QC], F32, tag="q",
                                              name="pqs", bufs=2)
                    nc.tensor.matmul(
                        side["pq"][:],
                        wq_sb[ct][:, jt * 128:(jt + 1) * 128],
                        side["xt"][ct][:],
                        start=(ct == 0), stop=(ct == CT - 1))
                    if ct == CT - 1:
                        nc.vector.tensor_scalar_add(
                            qk_sb[jt][:, qc * QC:(qc + 1) * QC],
                            side["pq"][:], bqk_sb[jt][:])

                attention(0, side_setup=side_setup, side_step=side_step)
                attention(1)

                if debug:
                    nc.sync.dma_start(dbg_qk[:], qk_sb[0][:])
                    nc.sync.dma_start(dbg_vh[:], vhat[:])
                    nc.sync.dma_start(dbg_sum[:], sums_all[:])
                    nc.sync.dma_start(dbg_au[:], attnU[0][:])

                # ---------- phase E: out projection (partial) ----------
                for mt in range(MT):
                    ob = dp.tile([128, C], F32, tag="o", name="ob", bufs=3)
                    for half in range(2):
                        po = psp.tile([128, 512], F32, tag="s", name="po")
                        for k in range(2):
                            nc.tensor.matmul(
                                po[:],
                                attnU[k][:, mt * 128:(mt + 1) * 128],
                                wout_sb[k][:, half * 512:(half + 1) * 512],
                                start=(k == 0), stop=(k == 1))
                        nc.vector.tensor_copy(ob[:, half * 512:(half + 1) * 512],
                                              po[:])
                    nc.sync.dma_start(outp_d[mt * 128:(mt + 1) * 128, :], ob[:])

    nc.compile()
    return nc


def shard_inputs(x, W_qkv, b_qkv, W_out, b_out):
    """Build per-core input maps. Column order inside a core's 768 qkv cols:
    [Q(h0)|Q(h1)|K(h0)|K(h1)|Q(h2)|Q(h3)|K(h2)|K(h3)|V(h0..h3)]."""
    import ml_dtypes
    x = np.asarray(x, np.float32)
    W_qkv = np.asarray(W_qkv, np.float32)
    b_qkv = np.asarray(b_qkv, np.float32)
    W_out = np.asarray(W_out, np.float32)
    xts = [np.ascontiguousarray(x[b].T) for b in range(B)]
    in_maps = []
    for core in range(NCORES):
        b, hg = divmod(core, HG)
        hs = [hg * HPC + i for i in range(HPC)]

        def qcol(h): return W_qkv[:, h * 3 * D: h * 3 * D + D]
        def kcol(h): return W_qkv[:, h * 3 * D + D: h * 3 * D + 2 * D]
        def vcol(h): return W_qkv[:, h * 3 * D + 2 * D: h * 3 * D + 3 * D]
        def qb(h): return b_qkv[h * 3 * D: h * 3 * D + D]
        def kb(h): return b_qkv[h * 3 * D + D: h * 3 * D + 2 * D]
        def vb(h): return b_qkv[h * 3 * D + 2 * D: h * 3 * D + 3 * D]

        wq_cols, bqk_parts = [], []
        for pg in range(2):
            h0, h1 = hs[2 * pg], hs[2 * pg + 1]
            wq_cols += [qcol(h0), qcol(h1), kcol(h0), kcol(h1)]
            bqk_parts += [qb(h0), qb(h1), kb(h0), kb(h1)]
        wq_cols += [vcol(h) for h in hs]
        wqkv_sh = np.ascontiguousarray(np.concatenate(wq_cols, axis=1))
        bqk_sh = np.ascontiguousarray(np.concatenate(bqk_parts))
        bv_sh = np.ascontiguousarray(np.concatenate([vb(h) for h in hs]))
        wout_sh = np.ascontiguousarray(
            np.concatenate([W_out[h * D:(h + 1) * D, :] for h in hs],
                           axis=0)).astype(ml_dtypes.bfloat16)
        in_maps.append({
            "xt": xts[b],
            "wqkv": wqkv_sh,
            "bqk": bqk_sh,
            "bv": bv_sh,
            "wout": wout_sh,
        })
    return in_maps


_NC_CACHE = {}


def _get_nc(loops=1):
    if loops not in _NC_CACHE:
        _NC_CACHE[loops] = build(loops)
    return _NC_CACHE[loops]


def kernel(x, W_qkv, b_qkv, W_out, b_out):
    from concourse.bass_utils import run_bass_kernel_spmd
    in_maps = shard_inputs(x, W_qkv, b_qkv, W_out, b_out)
    nc = _get_nc(1)
    r = run_bass_kernel_spmd(nc, in_maps, list(range(NCORES)), trace=False)
    b_out = np.asarray(b_out, np.float32)
    out = np.empty((B, T, C), np.float32)
    for b in range(B):
        acc = r.results[b * HG]["outp"].copy()
        for hg in range(1, HG):
            acc += r.results[b * HG + hg]["outp"]
        out[b] = acc + b_out
    return out
